# revision 27
# baseline (speedup 1.0000x reference)
import sys
if "/opt/trn_rl_repo" not in sys.path:
    sys.path.insert(0, "/opt/trn_rl_repo")

import numpy as np
import ml_dtypes
import concourse.bacc as bacc
import concourse.tile as tile
from concourse.tile_rust import add_dep_helper
from concourse import mybir
from concourse.bass_utils import run_bass_kernel_spmd

B, S, D = 4, 2048, 1024
NCORES = 8
F32 = mybir.dt.float32
F32R = mybir.dt.float32r
BF16 = mybir.dt.bfloat16
_cache = {}


def _build(reps=1):
    if reps in _cache:
        return _cache[reps]
    nc = bacc.Bacc()
    xt = nc.dram_tensor("xt", [D, B * S], BF16, kind="ExternalInput")
    wq = nc.dram_tensor("wq", [128, D], BF16, kind="ExternalInput")
    wk = nc.dram_tensor("wk", [128, D], BF16, kind="ExternalInput")
    wv = nc.dram_tensor("wv", [128, D], BF16, kind="ExternalInput")
    wo = nc.dram_tensor("wo", [128, D], F32R, kind="ExternalInput")
    bq = nc.dram_tensor("bq", [128, 1], F32, kind="ExternalInput")
    bk = nc.dram_tensor("bk", [128, 1], F32, kind="ExternalInput")
    on64 = nc.dram_tensor("on64", [128, 64], BF16, kind="ExternalInput")
    po = nc.dram_tensor("po", [B * D, S], F32, kind="ExternalOutput")
    warm = nc.dram_tensor("warm", [128, 512], F32, kind="ExternalOutput")

    ACT = mybir.ActivationFunctionType

    with tile.TileContext(nc) as tc:
        with tc.tile_pool(name="sb", bufs=1) as sb, \
             tc.tile_pool(name="ps", bufs=2, space="PSUM") as ps:
            wq_sb = sb.tile([128, D], BF16)
            wk_sb = sb.tile([128, D], BF16)
            wv_sb = sb.tile([128, D], BF16)
            wo_sb = sb.tile([128, D], F32R)
            bq_sb = sb.tile([128, 1], F32)
            bk_sb = sb.tile([128, 1], F32)
            nc.sync.dma_start(out=wq_sb, in_=wq[:, :])
            nc.sync.dma_start(out=wk_sb, in_=wk[:, :])
            nc.sync.dma_start(out=wv_sb, in_=wv[:, :])
            nc.sync.dma_start(out=wo_sb, in_=wo[:, :])
            nc.sync.dma_start(out=bq_sb, in_=bq[:, :])
            nc.sync.dma_start(out=bk_sb, in_=bk[:, :])

            # vp: 16 sk-tiles x (64 ones | 64 V_h0 | 64 ones | 64 V_h1) =
            # 256 cols, bf16.  PV lhsT for head h = cols [h*128:(h+1)*128]
            # = [1 | V_h]: the ones block rides along in the matmul and
            # lands the softmax denominator on PSUM rows 0:64 (a free
            # partition-broadcast), ctx on rows 64:128.  V blocks are
            # filled by DMA xbar transposes -- zero engine cost.
            vp = sb.tile([128, 16 * 256], BF16)
            for t in range(16):
                nc.sync.dma_start(
                    out=vp[:, t * 256:t * 256 + 64], in_=on64[:, :])
                nc.sync.dma_start(
                    out=vp[:, t * 256 + 128:t * 256 + 192], in_=on64[:, :])

            qt = [sb.tile([128, S], F32R, name=f"qt{i}") for i in range(2)]
            kt = [sb.tile([128, S], F32R, name=f"kt{i}") for i in range(2)]
            # v staging: the DVE flush lands in vstg; a plain SBUF->SBUF
            # DMA copies it into vt, because the xbar transpose misreads
            # engine-written bf16 sources (DMA-written sources are fine)
            vstg = [sb.tile([128, S], BF16, name=f"vstg{i}")
                    for i in range(2)]
            vt = [sb.tile([128, S], BF16, name=f"vt{i}") for i in range(2)]
            ctxT = sb.tile([128, S], F32R)
            ctxU = sb.tile([128, S], F32R)
            vt_copies = {0: [], 1: []}
            vdma_insts = {}
            pv_readers = {}

            def emit_xs(bi):
                xsl = []
                for k in range(8):
                    xs = sb.tile([128, S], BF16, tag="xs", bufs=8)
                    nc.sync.dma_start(
                        out=xs,
                        in_=xt[k * 128:(k + 1) * 128, bi * S:(bi + 1) * S])
                    xsl.append(xs)
                return xsl

            def emit_vdma(t, par):
                # V'_t = vt[:, t-block].T via DMA xbar transpose.  The
                # transpose APs are opaque to tile dep-tracking, so order
                # it explicitly after the vt fill and after this batch's
                # PV reads of the vp block it overwrites.
                d1 = nc.sync.dma_start_transpose(
                    out=vp[:, t * 256 + 64:t * 256 + 128],
                    in_=vt[par][0:64, t * 128:(t + 1) * 128])
                d2 = nc.sync.dma_start_transpose(
                    out=vp[:, t * 256 + 192:t * 256 + 256],
                    in_=vt[par][64:128, t * 128:(t + 1) * 128])
                for d in (d1, d2):
                    for c in vt_copies[par]:
                        add_dep_helper(d.ins, c.ins, True, "vdma-raw-vt")
                    for r in pv_readers.get(t, []):
                        add_dep_helper(d.ins, r.ins, True, "vdma-war-pv")
                vdma_insts[t] = (d1, d2)

            wbt = ((wq_sb, bq_sb), (wk_sb, bk_sb), (wv_sb, None))

            # one qkv projection matmul; groups of 16 accumulate into the
            # "aux" PSUM slot, flushed on DVE when the group completes
            qkv_state = {}

            def emit_qkv_mm(g, n2, k, xsl, par):
                proj, half = g // 2, g % 2
                wt, bt = wbt[proj]
                if (n2, k) == (0, 0):
                    qkv_state["pq"] = ps.tile([128, 1024], F32, tag="aux",
                                              bufs=1, name="pq")
                pq = qkv_state["pq"]
                c0 = half * 1024 + n2 * 512
                nc.tensor.matmul(
                    pq[:, n2 * 512:(n2 + 1) * 512],
                    wt[:, k * 128:(k + 1) * 128],
                    xsl[k][:, c0:c0 + 512],
                    start=(k == 0), stop=(k == 7))
                if (n2, k) == (1, 7):
                    dst = (qt, kt, vstg)[proj][par]
                    dsl = dst[:, half * 1024:(half + 1) * 1024]
                    if bt is None:
                        nc.vector.tensor_copy(out=dsl, in_=pq[:, :])
                        c = nc.sync.dma_start(
                            out=vt[par][:, half * 1024:(half + 1) * 1024],
                            in_=vstg[par][:, half * 1024:(half + 1) * 1024])
                        if half == 0:
                            vt_copies[par] = []
                        vt_copies[par].append(c)
                    else:
                        nc.vector.tensor_scalar_add(
                            out=dsl, in0=pq[:, :], scalar1=bt[:, 0:1])

            def emit_qkv_group(g, xsl, par):
                for n2 in range(2):
                    for k in range(8):
                        emit_qkv_mm(g, n2, k, xsl, par)

            seq = list(range(B)) * reps
            # prologue: x tiles take ~12us to DMA in; run dummy
            # accumulating matmuls meanwhile so the PE p-state ramp is
            # warm (2.4 GHz) by the time real work arrives.  The flush
            # that keeps them from being DCE'd is emitted at build end.
            xsl = emit_xs(seq[0])
            wp = ps.tile([128, 512], F32, tag="pa", bufs=2)
            for w in range(28):
                nc.tensor.matmul(wp, wo_sb[:, 0:128], wo_sb[:, 0:512],
                                 start=(w == 0), stop=(w == 27))
            # flush now to free the PSUM slot; the DMA that makes this
            # observable (anti-DCE) is emitted at build end so it doesn't
            # block the x-tile DMAs on the sync queue.
            wfl = sb.tile([128, 512], F32)
            nc.vector.tensor_copy(out=wfl, in_=wp[:, :])
            for g in range(6):
                emit_qkv_group(g, xsl, 0)

            first = True
            for i, b in enumerate(seq):
                par = i % 2
                nxt = seq[(i + 1) % len(seq)]
                last = i + 1 == len(seq)
                xsl = emit_xs(nxt)

                if first:
                    for t in range(16):
                        emit_vdma(t, par)
                    first = False

                # qkv for the next batch, spread MM-by-MM across the
                # stage loop so the PE never idles while ACT works
                # through the exp stream (only 3 groups' worth on the
                # last batch -- just enough to cover the softmax tails).
                qmms = []
                for g in range(3 if last else 6):
                    for n2 in range(2):
                        for k in range(8):
                            qmms.append((g, n2, k))
                qemit = 0

                ets = {}
                cxps = {}
                bcss = {}

                def emit_outproj(g16, tag):
                    half, m = g16 // 8, g16 % 8
                    pso = ps.tile([128, 1024], F32, tag=tag,
                                  bufs=(2 if tag == "pa" else 1))
                    for n2 in range(2):
                        c0 = half * 1024 + n2 * 512
                        nc.tensor.matmul(
                            pso[:, n2 * 512:(n2 + 1) * 512],
                            wo_sb[:, m * 128:(m + 1) * 128],
                            ctxT[:, c0:c0 + 512], start=True, stop=True)
                    ob = sb.tile([128, 1024], F32, tag="ob", bufs=4)
                    if g16 % 2 == 0:
                        nc.vector.tensor_copy(out=ob, in_=pso[:, :])
                    else:
                        nc.scalar.copy(out=ob, in_=pso[:, :])
                    nc.sync.dma_start(
                        out=po[b * D + m * 128:b * D + (m + 1) * 128,
                               half * 1024:(half + 1) * 1024],
                        in_=ob)

                def emit_tt(p1):
                    # deferred normalize: ctxT = ctxU * (1/Z), SBUF-only
                    h1, j1 = p1 // 2, p1 % 2
                    r0 = h1 * 64
                    if h1 == 1:
                        # verifier needs equal SBUF base partitions:
                        # mirror the reciprocal rows down to base 64
                        nc.vector.tensor_copy(
                            out=bcss[p1][64:128, :], in_=bcss[p1][0:64, :])
                    nc.vector.tensor_tensor(
                        ctxT[r0:r0 + 64, j1 * 1024:(j1 + 1) * 1024],
                        ctxU[r0:r0 + 64, j1 * 1024:(j1 + 1) * 1024],
                        bcss[p1][r0:r0 + 64, :], mybir.AluOpType.mult)

                for s in range(66):
                    if s < 64:
                        p, t = s // 16, s % 16
                        h, j = p // 2, p % 2
                        scp = ps.tile([128, 1024], F32, tag="pa", bufs=2)
                        for n2 in range(2):
                            q0 = j * 1024 + n2 * 512
                            nc.tensor.matmul(
                                scp[:, n2 * 512:(n2 + 1) * 512],
                                kt[par][h * 64:(h + 1) * 64,
                                        t * 128:(t + 1) * 128],
                                qt[par][h * 64:(h + 1) * 64, q0:q0 + 512],
                                start=True, stop=True)
                        et = sb.tile([128, 1024], BF16, tag="et", bufs=4)
                        nc.scalar.activation(
                            out=et, in_=scp, func=ACT.Exp, scale=0.125)
                        ets[s] = et
                    while qemit < len(qmms) and \
                            qemit < (s + 1) * len(qmms) // 60:
                        g, n2, k = qmms[qemit]
                        emit_qkv_mm(g, n2, k, xsl, 1 - par)
                        qemit += 1
                    if s >= 2:
                        p1, t1 = (s - 2) // 16, (s - 2) % 16
                        h1 = p1 // 2
                        if t1 == 0:
                            cxps[p1] = ps.tile([128, 1024], F32, tag="cx",
                                               bufs=1, name="cxp")
                        et1 = ets.pop(s - 2)
                        for n2 in range(2):
                            pv = nc.tensor.matmul(
                                cxps[p1][:, n2 * 512:(n2 + 1) * 512],
                                vp[:, t1 * 256 + h1 * 128:
                                   t1 * 256 + (h1 + 1) * 128],
                                et1[:, n2 * 512:(n2 + 1) * 512],
                                start=(t1 == 0), stop=(t1 == 15))
                            for d in vdma_insts.get(t1, ()):
                                add_dep_helper(pv.ins, d.ins, True,
                                               "pv-raw-vdma")
                            if p1 == 0 and n2 == 0:
                                pv_readers[t1] = []
                            pv_readers[t1].append(pv)
                        if t1 == 15:
                            j1 = p1 % 2
                            cxp = cxps.pop(p1)
                            # free the accumulator fast: reciprocal of the
                            # Z rows on DVE concurrently with the raw ctx
                            # copy on ACT; the normalize multiply is
                            # deferred off the critical path.
                            bcs = sb.tile([128, 1024], F32, tag="bcs",
                                          bufs=4)
                            bcss[p1] = bcs
                            nc.vector.reciprocal_approx_fast(
                                out=bcs[0:64, :], in_=cxp[0:64, :])
                            nc.scalar.copy(
                                out=ctxU[h1 * 64:(h1 + 1) * 64,
                                         j1 * 1024:(j1 + 1) * 1024],
                                in_=cxp[64:128, :])
                    if s == 30:
                        emit_tt(0)
                    elif s == 46:
                        emit_tt(1)
                    elif s == 53:
                        emit_tt(2)
                    # V' transposes for the next batch ride on the DMA
                    # engines; vt halves land after the g4/g5 flushes.
                    # Block t's last in-batch PV read is at stage 50+t,
                    # so t can go from stage 53+t (t=12..15 after the
                    # loop, past their stage-63..65 readers).
                    if not last and 53 <= s <= 64:
                        emit_vdma(s - 53, 1 - par)
                    # out-proj half=0 starts once p2's normalize is done
                    # and the qkv stream has vacated the "aux" slot
                    if 61 <= s <= 65:
                        emit_outproj(s - 61, "aux")

                if not last:
                    for t4 in range(12, 16):
                        emit_vdma(t4, 1 - par)
                emit_tt(3)
                for gi, g16 in enumerate(range(5, 16)):
                    tag = "cx" if gi % 3 == 2 else "pa"
                    emit_outproj(g16, tag)

            # anti-DCE output for the warmup matmuls
            nc.sync.dma_start(out=warm[:, :], in_=wfl)
    nc.finalize()
    _cache[reps] = nc
    return nc


def _warr(W):
    # W [128 outdims, 1024 indims] -> SBUF lhsT layout [128 p, 8k x 128 m]
    return np.ascontiguousarray(
        W.reshape(128, 8, 128).transpose(2, 1, 0).reshape(128, 1024))


def _in_maps(x, qkv_w, qkv_b, out_w):
    xT = np.ascontiguousarray(
        x.reshape(B * S, D).T).astype(ml_dtypes.bfloat16)
    in_maps = []
    for c in range(NCORES):
        base = c * 128
        V = out_w[:, base:base + 128]
        in_maps.append({
            "xt": xT,
            "wq": _warr(qkv_w[base:base + 128, :]).astype(ml_dtypes.bfloat16),
            "wk": _warr(qkv_w[D + base:D + base + 128, :]).astype(ml_dtypes.bfloat16),
            "wv": _warr(qkv_w[2 * D + base:2 * D + base + 128, :]).astype(ml_dtypes.bfloat16),
            "wo": np.ascontiguousarray(
                V.reshape(8, 128, 128).transpose(2, 0, 1).reshape(128, 1024)
            ).astype(np.float32),
            "on64": np.ones((128, 64), dtype=ml_dtypes.bfloat16),
            "bq": qkv_b[base:base + 128].reshape(128, 1).astype(np.float32),
            "bk": qkv_b[D + base:D + base + 128].reshape(128, 1).astype(np.float32),
        })
    return in_maps


def kernel(x, qkv_w, qkv_b, out_w, out_b):
    nc = _build()
    in_maps = _in_maps(x, qkv_w, qkv_b, out_w)
    res = run_bass_kernel_spmd(nc, in_maps, core_ids=list(range(NCORES)),
                               trace=False)
    kernel.last_exec_ns = res.exec_time_ns
    acc = np.zeros((B, D, S), dtype=np.float64)
    for c in range(NCORES):
        acc += res.results[c]["po"].reshape(B, D, S)
    # v-bias folds into the output bias: ctx = attn@(v+bv) = attn@v + bv
    # (attention rows sum to 1), so out += out_w @ bv is exact.
    out_b_eff = out_b.astype(np.float64) + \
        out_w.astype(np.float64) @ qkv_b[2 * D:3 * D].astype(np.float64)
    out = acc.transpose(0, 2, 1) + out_b_eff
    return out.astype(np.float32)


# revision 32
# speedup vs baseline: 1.1230x; 1.1230x over previous
import sys
if "/opt/trn_rl_repo" not in sys.path:
    sys.path.insert(0, "/opt/trn_rl_repo")

import numpy as np
import ml_dtypes
import concourse.bacc as bacc
import concourse.tile as tile
from concourse.tile_rust import add_dep_helper
from concourse import mybir
from concourse.bass_utils import run_bass_kernel_spmd

B, S, D = 4, 2048, 1024
NCORES = 8
F32 = mybir.dt.float32
F32R = mybir.dt.float32r
BF16 = mybir.dt.bfloat16
_cache = {}


def _build(reps=1):
    if reps in _cache:
        return _cache[reps]
    nc = bacc.Bacc()
    xt = nc.dram_tensor("xt", [D, B * S], BF16, kind="ExternalInput")
    wq = nc.dram_tensor("wq", [128, D], BF16, kind="ExternalInput")
    wk = nc.dram_tensor("wk", [128, D], BF16, kind="ExternalInput")
    wv = nc.dram_tensor("wv", [128, D], BF16, kind="ExternalInput")
    wo = nc.dram_tensor("wo", [128, D], F32R, kind="ExternalInput")
    bq = nc.dram_tensor("bq", [128, 1], F32, kind="ExternalInput")
    bk = nc.dram_tensor("bk", [128, 1], F32, kind="ExternalInput")
    on64 = nc.dram_tensor("on64", [128, 64], BF16, kind="ExternalInput")
    po = nc.dram_tensor("po", [B * D, S], F32, kind="ExternalOutput")
    warm = nc.dram_tensor("warm", [128, 512], F32, kind="ExternalOutput")

    ACT = mybir.ActivationFunctionType

    with tile.TileContext(nc) as tc:
        with tc.tile_pool(name="sb", bufs=1) as sb, \
             tc.tile_pool(name="ps", bufs=2, space="PSUM") as ps:
            wq_sb = sb.tile([128, D], BF16)
            wk_sb = sb.tile([128, D], BF16)
            wv_sb = sb.tile([128, D], BF16)
            wo_sb = sb.tile([128, D], F32R)
            bq_sb = sb.tile([128, 1], F32)
            bk_sb = sb.tile([128, 1], F32)
            nc.sync.dma_start(out=wq_sb, in_=wq[:, :])
            nc.sync.dma_start(out=wk_sb, in_=wk[:, :])
            nc.sync.dma_start(out=wv_sb, in_=wv[:, :])
            nc.sync.dma_start(out=wo_sb, in_=wo[:, :])
            nc.sync.dma_start(out=bq_sb, in_=bq[:, :])
            nc.sync.dma_start(out=bk_sb, in_=bk[:, :])

            # vp: 16 sk-tiles x (64 ones | 64 V_h0 | 64 ones | 64 V_h1) =
            # 256 cols, bf16.  PV lhsT for head h = cols [h*128:(h+1)*128]
            # = [1 | V_h]: the ones block rides along in the matmul and
            # lands the softmax denominator on PSUM rows 0:64 (a free
            # partition-broadcast), ctx on rows 64:128.  V blocks are
            # filled by DMA xbar transposes -- zero engine cost.
            vp = sb.tile([128, 16 * 256], BF16)
            for t in range(16):
                nc.sync.dma_start(
                    out=vp[:, t * 256:t * 256 + 64], in_=on64[:, :])
                nc.sync.dma_start(
                    out=vp[:, t * 256 + 128:t * 256 + 192], in_=on64[:, :])

            qt = [sb.tile([128, S], F32R, name=f"qt{i}") for i in range(2)]
            kt = [sb.tile([128, S], F32R, name=f"kt{i}") for i in range(2)]
            # v staging: the DVE flush lands in vstg; a plain SBUF->SBUF
            # DMA copies it into vt, because the xbar transpose misreads
            # engine-written bf16 sources (DMA-written sources are fine)
            vstg = [sb.tile([128, S], BF16, name=f"vstg{i}")
                    for i in range(2)]
            vt = [sb.tile([128, S], BF16, name=f"vt{i}") for i in range(2)]
            ctxT = sb.tile([128, S], F32R)
            ctxU = sb.tile([128, S], F32R)
            vt_copies = {0: [], 1: []}
            vdma_insts = {}
            pv_readers = {}

            def emit_xs(bi):
                xsl = []
                for k in range(8):
                    xs = sb.tile([128, S], BF16, tag="xs", bufs=8)
                    nc.sync.dma_start(
                        out=xs,
                        in_=xt[k * 128:(k + 1) * 128, bi * S:(bi + 1) * S])
                    xsl.append(xs)
                return xsl

            def emit_vdma(t, par):
                # V'_t = vt[:, t-block].T via DMA xbar transpose.  The
                # transpose APs are opaque to tile dep-tracking, so order
                # it explicitly after the vt fill and after this batch's
                # PV reads of the vp block it overwrites.
                d1 = nc.sync.dma_start_transpose(
                    out=vp[:, t * 256 + 64:t * 256 + 128],
                    in_=vt[par][0:64, t * 128:(t + 1) * 128])
                d2 = nc.sync.dma_start_transpose(
                    out=vp[:, t * 256 + 192:t * 256 + 256],
                    in_=vt[par][64:128, t * 128:(t + 1) * 128])
                for d in (d1, d2):
                    for c in vt_copies[par]:
                        add_dep_helper(d.ins, c.ins, True, "vdma-raw-vt")
                    r = pv_readers.get(t)
                    if r is not None:
                        # last PV reader of this vp block; the in-order PE
                        # queue makes earlier readers safe transitively
                        add_dep_helper(d.ins, r.ins, True, "vdma-war-pv")
                vdma_insts[t] = (d1, d2)

            wbt = ((wq_sb, bq_sb), (wk_sb, bk_sb), (wv_sb, None))

            # one qkv projection matmul; groups of 16 accumulate into the
            # "aux" PSUM slot, flushed on DVE when the group completes
            qkv_state = {}

            def emit_qkv_mm(g, n2, k, xsl, par):
                proj, half = g // 2, g % 2
                wt, bt = wbt[proj]
                if (n2, k) == (0, 0):
                    qkv_state["pq"] = ps.tile([128, 1024], F32, tag="aux",
                                              bufs=1, name="pq")
                pq = qkv_state["pq"]
                c0 = half * 1024 + n2 * 512
                nc.tensor.matmul(
                    pq[:, n2 * 512:(n2 + 1) * 512],
                    wt[:, k * 128:(k + 1) * 128],
                    xsl[k][:, c0:c0 + 512],
                    start=(k == 0), stop=(k == 7))
                if (n2, k) == (1, 7):
                    dst = (qt, kt, vstg)[proj][par]
                    dsl = dst[:, half * 1024:(half + 1) * 1024]
                    if bt is None:
                        nc.vector.tensor_copy(out=dsl, in_=pq[:, :])
                        # SWDGE (gpsimd) queue: its sem wait on the DVE
                        # flush must not block the sync queue's x-loads
                        c = nc.gpsimd.dma_start(
                            out=vt[par][:, half * 1024:(half + 1) * 1024],
                            in_=vstg[par][:, half * 1024:(half + 1) * 1024])
                        if half == 0:
                            vt_copies[par] = []
                        vt_copies[par].append(c)
                    else:
                        nc.vector.tensor_scalar_add(
                            out=dsl, in0=pq[:, :], scalar1=bt[:, 0:1])

            def emit_qkv_group(g, xsl, par):
                for n2 in range(2):
                    for k in range(8):
                        emit_qkv_mm(g, n2, k, xsl, par)

            seq = list(range(B)) * reps
            # prologue: x tiles take ~12us to DMA in; run dummy
            # accumulating matmuls meanwhile so the PE p-state ramp is
            # warm (2.4 GHz) by the time real work arrives.  The flush
            # that keeps them from being DCE'd is emitted at build end.
            xsl = emit_xs(seq[0])
            wp = ps.tile([128, 512], F32, tag="pa", bufs=2)
            for w in range(28):
                nc.tensor.matmul(wp, wo_sb[:, 0:128], wo_sb[:, 0:512],
                                 start=(w == 0), stop=(w == 27))
            # flush now to free the PSUM slot; the DMA that makes this
            # observable (anti-DCE) is emitted at build end so it doesn't
            # block the x-tile DMAs on the sync queue.
            wfl = sb.tile([128, 512], F32)
            nc.vector.tensor_copy(out=wfl, in_=wp[:, :])
            for g in range(6):
                emit_qkv_group(g, xsl, 0)

            first = True
            for i, b in enumerate(seq):
                par = i % 2
                nxt = seq[(i + 1) % len(seq)]
                last = i + 1 == len(seq)
                xsl = emit_xs(nxt)

                if first:
                    for t in range(16):
                        emit_vdma(t, par)
                    first = False
                else:
                    # the tail vp blocks for THIS batch: emitted here (not
                    # at the previous batch's end) so their sem waits are
                    # satisfied at issue and never stall the sync queue
                    for t4 in range(12, 16):
                        emit_vdma(t4, par)

                # qkv for the next batch, spread MM-by-MM across the
                # stage loop so the PE never idles while ACT works
                # through the exp stream (only 3 groups' worth on the
                # last batch -- just enough to cover the softmax tails).
                qmms = []
                for g in range(3 if last else 6):
                    for n2 in range(2):
                        for k in range(8):
                            qmms.append((g, n2, k))
                qemit = 0

                ets = {}
                cxps = {}
                bcss = {}

                def emit_outproj(g16, tag):
                    half, m = g16 // 8, g16 % 8
                    pso = ps.tile([128, 1024], F32, tag=tag,
                                  bufs=(2 if tag == "pa" else 1))
                    for n2 in range(2):
                        c0 = half * 1024 + n2 * 512
                        nc.tensor.matmul(
                            pso[:, n2 * 512:(n2 + 1) * 512],
                            wo_sb[:, m * 128:(m + 1) * 128],
                            ctxT[:, c0:c0 + 512], start=True, stop=True)
                    ob = sb.tile([128, 1024], F32, tag="ob", bufs=4)
                    if g16 % 2 == 0:
                        nc.vector.tensor_copy(out=ob, in_=pso[:, :])
                    else:
                        nc.scalar.copy(out=ob, in_=pso[:, :])
                    nc.sync.dma_start(
                        out=po[b * D + m * 128:b * D + (m + 1) * 128,
                               half * 1024:(half + 1) * 1024],
                        in_=ob)

                def emit_tt(p1):
                    # deferred normalize: ctxT = ctxU * (1/Z), SBUF-only
                    h1, j1 = p1 // 2, p1 % 2
                    r0 = h1 * 64
                    if h1 == 1:
                        # verifier needs equal SBUF base partitions:
                        # mirror the reciprocal rows down to base 64
                        nc.vector.tensor_copy(
                            out=bcss[p1][64:128, :], in_=bcss[p1][0:64, :])
                    nc.vector.tensor_tensor(
                        ctxT[r0:r0 + 64, j1 * 1024:(j1 + 1) * 1024],
                        ctxU[r0:r0 + 64, j1 * 1024:(j1 + 1) * 1024],
                        bcss[p1][r0:r0 + 64, :], mybir.AluOpType.mult)

                for s in range(66):
                    if s < 64:
                        p, t = s // 16, s % 16
                        h, j = p // 2, p % 2
                        scp = ps.tile([128, 1024], F32, tag="pa", bufs=2)
                        for n2 in range(2):
                            q0 = j * 1024 + n2 * 512
                            nc.tensor.matmul(
                                scp[:, n2 * 512:(n2 + 1) * 512],
                                kt[par][h * 64:(h + 1) * 64,
                                        t * 128:(t + 1) * 128],
                                qt[par][h * 64:(h + 1) * 64, q0:q0 + 512],
                                start=True, stop=True)
                        et = sb.tile([128, 1024], BF16, tag="et", bufs=4)
                        nc.scalar.activation(
                            out=et, in_=scp, func=ACT.Exp, scale=0.125)
                        ets[s] = et
                    while qemit < len(qmms) and \
                            qemit < (s + 1) * len(qmms) // 60:
                        g, n2, k = qmms[qemit]
                        emit_qkv_mm(g, n2, k, xsl, 1 - par)
                        qemit += 1
                    if s >= 2:
                        p1, t1 = (s - 2) // 16, (s - 2) % 16
                        h1 = p1 // 2
                        if t1 == 0:
                            cxps[p1] = ps.tile([128, 1024], F32, tag="cx",
                                               bufs=1, name="cxp")
                        et1 = ets.pop(s - 2)
                        for n2 in range(2):
                            pv = nc.tensor.matmul(
                                cxps[p1][:, n2 * 512:(n2 + 1) * 512],
                                vp[:, t1 * 256 + h1 * 128:
                                   t1 * 256 + (h1 + 1) * 128],
                                et1[:, n2 * 512:(n2 + 1) * 512],
                                start=(t1 == 0), stop=(t1 == 15))
                            if p1 == 0 and n2 == 0:
                                # first reader orders the whole in-order
                                # PE stream after the vdma write
                                for d in vdma_insts.get(t1, ()):
                                    add_dep_helper(pv.ins, d.ins, True,
                                                   "pv-raw-vdma")
                            pv_readers[t1] = pv
                        if t1 == 15:
                            j1 = p1 % 2
                            cxp = cxps.pop(p1)
                            # free the accumulator fast: reciprocal of the
                            # Z rows on DVE concurrently with the raw ctx
                            # copy on ACT; the normalize multiply is
                            # deferred off the critical path.
                            bcs = sb.tile([128, 1024], F32, tag="bcs",
                                          bufs=4)
                            bcss[p1] = bcs
                            nc.vector.reciprocal_approx_fast(
                                out=bcs[0:64, :], in_=cxp[0:64, :])
                            nc.scalar.copy(
                                out=ctxU[h1 * 64:(h1 + 1) * 64,
                                         j1 * 1024:(j1 + 1) * 1024],
                                in_=cxp[64:128, :])
                    if s == 30:
                        emit_tt(0)
                    elif s == 46:
                        emit_tt(1)
                    elif s == 53:
                        emit_tt(2)
                    # V' transposes for the next batch ride on the DMA
                    # engines; vt halves land after the g4/g5 flushes.
                    # Block t's last in-batch PV read is at stage 50+t,
                    # so t can go from stage 53+t (t=12..15 after the
                    # loop, past their stage-63..65 readers).
                    if not last and 53 <= s <= 64:
                        emit_vdma(s - 53, 1 - par)
                    # out-proj half=0 starts once p2's normalize is done
                    # and the qkv stream has vacated the "aux" slot
                    if 61 <= s <= 65:
                        emit_outproj(s - 61, "aux")

                emit_tt(3)
                for gi, g16 in enumerate(range(5, 16)):
                    tag = "cx" if gi % 3 == 2 else "pa"
                    emit_outproj(g16, tag)

            # anti-DCE output for the warmup matmuls
            nc.sync.dma_start(out=warm[:, :], in_=wfl)
    nc.finalize()
    _cache[reps] = nc
    return nc


def _warr(W):
    # W [128 outdims, 1024 indims] -> SBUF lhsT layout [128 p, 8k x 128 m]
    return np.ascontiguousarray(
        W.reshape(128, 8, 128).transpose(2, 1, 0).reshape(128, 1024))


def _in_maps(x, qkv_w, qkv_b, out_w):
    xT = np.ascontiguousarray(
        x.reshape(B * S, D).T).astype(ml_dtypes.bfloat16)
    in_maps = []
    for c in range(NCORES):
        base = c * 128
        V = out_w[:, base:base + 128]
        in_maps.append({
            "xt": xT,
            "wq": _warr(qkv_w[base:base + 128, :]).astype(ml_dtypes.bfloat16),
            "wk": _warr(qkv_w[D + base:D + base + 128, :]).astype(ml_dtypes.bfloat16),
            "wv": _warr(qkv_w[2 * D + base:2 * D + base + 128, :]).astype(ml_dtypes.bfloat16),
            "wo": np.ascontiguousarray(
                V.reshape(8, 128, 128).transpose(2, 0, 1).reshape(128, 1024)
            ).astype(np.float32),
            "on64": np.ones((128, 64), dtype=ml_dtypes.bfloat16),
            "bq": qkv_b[base:base + 128].reshape(128, 1).astype(np.float32),
            "bk": qkv_b[D + base:D + base + 128].reshape(128, 1).astype(np.float32),
        })
    return in_maps


def kernel(x, qkv_w, qkv_b, out_w, out_b):
    nc = _build()
    in_maps = _in_maps(x, qkv_w, qkv_b, out_w)
    res = run_bass_kernel_spmd(nc, in_maps, core_ids=list(range(NCORES)),
                               trace=False)
    kernel.last_exec_ns = res.exec_time_ns
    acc = np.zeros((B, D, S), dtype=np.float64)
    for c in range(NCORES):
        acc += res.results[c]["po"].reshape(B, D, S)
    # v-bias folds into the output bias: ctx = attn@(v+bv) = attn@v + bv
    # (attention rows sum to 1), so out += out_w @ bv is exact.
    out_b_eff = out_b.astype(np.float64) + \
        out_w.astype(np.float64) @ qkv_b[2 * D:3 * D].astype(np.float64)
    out = acc.transpose(0, 2, 1) + out_b_eff
    return out.astype(np.float32)
